# revision 15
# baseline (speedup 1.0000x reference)
"""Trainium2 Bass kernel for nn_ChordalPCWeightTransform.

Math: the reference does
    out = softmax( P_orig( P_rootfirst(x) * w ), axis=-1 )
where P_rootfirst / P_orig are per-label rolls of the first 12 pitch
classes (last slot fixed).  The two permutations are exact inverses, so
the whole transform collapses to
    out[b, l, :] = softmax( x[b, l, :] * W[l, :] )
with W[l, j] = w[(j - root_pc(l)) % 12] for j < 12 and W[l, 12] = w[12].
W ([144, 13]) is a cheap host-side gather of the 13 learned weights.

v3: bf16 I/O (tolerance is 2e-2; bf16 keeps us ~1e-2) halves HBM traffic
vs the f32 baseline (123 MB -> 61 MB per core), moving the memory
roofline from ~343us to ~171us.  Compute is spread across engines with
measured rates (DVE 2x for dense bf16 tensor_tensor, 1x for reduce and
broadcast ops; GPSIMD ~2.5 cyc/elem):
  DMA in (HWDGE/sync, bf16)
  DVE:  t = x * W          (bf16 dense, 2x_1P)
  ACT:  e = exp(t)         (bf16 out)
  DVE:  s = reduce_sum over 13  (f32 out, 1x -- single op beats an add
        tree because every extra DVE op pays a ~0.4us DRAIN)
  ACT:  ls = ln(s); r = exp(-ls) = 1/s   (same ACT table set as exp)
  GPSIMD (6 of 7 tiles) / DVE (1 of 7): out = e * r (broadcast over 13)
  DMA out (HWDGE/scalar, bf16)
Host upcasts the bf16 result to f32.
"""

import numpy as np
import ml_dtypes

import concourse.bass as bass
import concourse.bacc as bacc
import concourse.tile as tile
from concourse import mybir
from concourse.bass_utils import run_bass_kernel_spmd

B, L, P = 65536, 144, 13
NCORES = 8
BS = B // NCORES   # 8192 frames per core
ROW = L * P        # 1872 floats per frame
TP = 128           # SBUF partitions
FPB = 4            # frames per partition per tile
TFREE = FPB * ROW  # free-dim elems per tile (x / e / out tiles)
K = FPB * L        # softmax groups per partition per tile (576)

# Of every DVE_FINAL_EVERY tiles, DVE does 1 final broadcast-mult and
# GPSIMD does the rest.  0 means GPSIMD does all of them.
DVE_FINAL_EVERY = 8

F32 = mybir.dt.float32
BF16 = mybir.dt.bfloat16


def _build_weight_table(w: np.ndarray) -> np.ndarray:
    """Effective per-label weight table W[l, j] = w[idx_original[l, j]]."""
    num_quality = L // 12
    root_pc = np.arange(L) // num_quality
    n = P - 1
    j = np.arange(n)
    idx12 = (j[None, :] - root_pc[:, None]) % n
    idx = np.concatenate([idx12, np.full((L, 1), n, dtype=idx12.dtype)], axis=1)
    return np.ascontiguousarray(w.astype(np.float32)[idx])  # [144, 13]


def _pin_act_table(nc) -> None:
    """Make Exp and Ln resolvable only from the combined set so Bacc emits a
    single ACT_TABLE_LOAD instead of thrashing exp<->ln sets every tile."""
    from concourse.hw_specs import get_activation_tables

    tabs = get_activation_tables(nc.m.arch)
    keep = "natural_log_exp_and_others"
    if keep not in tabs:
        return
    exp = mybir.ActivationFunctionType.Exp
    ln = mybir.ActivationFunctionType.Ln
    for name, fns in tabs.items():
        if name != keep:
            fns.discard(exp)
            fns.discard(ln)


def build_module(n_frames: int = BS) -> bass.Bass:
    tile_frames = TP * FPB
    assert n_frames % tile_frames == 0
    nt = n_frames // tile_frames
    nc = bacc.Bacc()
    _pin_act_table(nc)
    x_in = nc.declare_dram_parameter("x", [n_frames, ROW], BF16, isOutput=False)
    w_in = nc.declare_dram_parameter("w", [ROW], BF16, isOutput=False)
    y_out = nc.declare_dram_parameter("y", [n_frames, ROW], BF16, isOutput=True)
    # Per-tile view: partition p holds FPB consecutive frames, contiguous.
    x_v = x_in.rearrange("(n p f) r -> n p (f r)", p=TP, f=FPB)
    y_v = y_out.rearrange("(n p f) r -> n p (f r)", p=TP, f=FPB)

    with tile.TileContext(nc) as tc:
        with (
            tc.tile_pool(name="singles", bufs=1) as singles,
            tc.tile_pool(name="xin", bufs=3) as xpool,
            tc.tile_pool(name="etile", bufs=4) as epool,
            tc.tile_pool(name="rexp", bufs=3) as rpool,
            tc.tile_pool(name="halves", bufs=2) as hpool,
            tc.tile_pool(name="stats", bufs=4) as spool,
        ):
            # W row replicated across partitions; broadcast across the FPB
            # frame slots via a stride-0 middle axis in the multiply below
            # (innermost step stays 1, so DVE 2x_1P is preserved).
            wb = singles.tile([TP, ROW], BF16)
            nc.gpsimd.dma_start(
                out=wb[:],
                in_=w_in[None, :].to_broadcast([TP, ROW]),
            )

            for i in range(nt):
                x_t = xpool.tile([TP, TFREE], BF16)
                nc.sync.dma_start(out=x_t[:], in_=x_v[i])

                # t = x * W  (bf16 dense, DVE 2x_1P)
                x3w = x_t.rearrange("p (f r) -> p f r", r=ROW)
                nc.vector.tensor_tensor(
                    out=x3w, in0=x3w,
                    in1=wb[:, None, :].to_broadcast([TP, FPB, ROW]),
                    op=mybir.AluOpType.mult,
                )

                # e = exp(t)
                e_t = epool.tile([TP, TFREE], BF16)
                nc.scalar.activation(
                    out=e_t[:], in_=x_t[:],
                    func=mybir.ActivationFunctionType.Exp,
                )

                # s[p, g] = sum_j e[p, g, j], split across engines:
                # GPSIMD folds the first 12 lanes in half (6-wide add),
                # DVE reduces the 6 and adds the 13th lane.  This moves
                # ~45% of the reduce off DVE onto the otherwise-idle GPSIMD.
                e3 = e_t.rearrange("p (g d) -> p g d", d=P)
                a_t = hpool.tile([TP, K * 6], BF16)
                a3 = a_t.rearrange("p (g d) -> p g d", d=6)
                nc.gpsimd.tensor_tensor(
                    out=a3, in0=e3[:, :, 0:6], in1=e3[:, :, 6:12],
                    op=mybir.AluOpType.add,
                )
                s_t = spool.tile([TP, K], F32)
                nc.vector.reduce_sum(
                    out=s_t[:], in_=a3, axis=mybir.AxisListType.X
                )
                nc.vector.tensor_tensor(
                    out=s_t[:], in0=s_t[:], in1=e3[:, :, 12],
                    op=mybir.AluOpType.add,
                )

                # ls = ln(s); rex = exp(-ls) = 1/s, broadcast-EXPANDED to a
                # dense [TP, K, 13] bf16 tile by reading ls with a stride-0
                # inner axis (ACT is 1 elem/cycle regardless of strides, and
                # it shares no SBUF port with DVE).  The final multiply then
                # runs dense bf16 on DVE at 2x with no GPSIMD contention.
                nc.scalar.activation(
                    out=s_t[:], in_=s_t[:],
                    func=mybir.ActivationFunctionType.Ln,
                )
                rex = rpool.tile([TP, TFREE], BF16)
                rex3 = rex.rearrange("p (g d) -> p g d", d=P)
                nc.scalar.activation(
                    out=rex3, in_=s_t[:, :, None].to_broadcast([TP, K, P]),
                    func=mybir.ActivationFunctionType.Exp, scale=-1.0,
                )

                # out = e * rex, in place into the e tile (dense, DVE 2x).
                nc.vector.tensor_tensor(
                    out=e_t[:], in0=e_t[:], in1=rex[:],
                    op=mybir.AluOpType.mult,
                )

                nc.sync.dma_start(out=y_v[i], in_=e_t[:])

    nc.finalize()
    return nc


_MODULE_CACHE: dict[int, bass.Bass] = {}


def _get_module(n_frames: int = BS) -> bass.Bass:
    if n_frames not in _MODULE_CACHE:
        _MODULE_CACHE[n_frames] = build_module(n_frames)
    return _MODULE_CACHE[n_frames]


def make_in_maps(x: np.ndarray, w: np.ndarray) -> list[dict[str, np.ndarray]]:
    weff = _build_weight_table(w).reshape(ROW).astype(ml_dtypes.bfloat16)
    xb = np.ascontiguousarray(x.reshape(B, ROW)).astype(ml_dtypes.bfloat16)
    return [
        {"x": xb[i * BS : (i + 1) * BS], "w": weff}
        for i in range(NCORES)
    ]


def kernel(**inputs: np.ndarray) -> np.ndarray:
    x = np.asarray(inputs["chordal_pc_vector"], dtype=np.float32)
    w = np.asarray(inputs["scale_degree_weight"], dtype=np.float32)
    assert x.shape == (B, L, P), x.shape

    nc = _get_module()
    in_maps = make_in_maps(x, w)
    res = run_bass_kernel_spmd(nc, in_maps, core_ids=list(range(NCORES)))
    out = np.concatenate(
        [
            np.asarray(res.results[i]["y"]).astype(np.float32).reshape(BS, L, P)
            for i in range(NCORES)
        ],
        axis=0,
    )
    return out
